# revision 13
# baseline (speedup 1.0000x reference)
"""Trainium2 Bass kernel for nn_Block_34067680592489.

Computes, for B=32768 independent signals x[b] (length 256):
  mu,reg = small-CNN(x[b])      (conv5+avgpool4+softplus twice, linear, softplus)
  grad   = TtT x - x_b + reg * DtD x
  x_t    = x - gamma * grad,  gamma = softplus(gamma_p)
  out    = middle root of z^3 -(m+x_t) z^2 + (m x_t - 2 gm) z + gm m,  gm = gamma*mu

Device algorithm (per element, after normalizing the cubic to mass=1):
  s   = (1 + xt)/3                    (from PE matmuls, accumulated in PSUM)
  P1' = (s - 1/2)^2                   (ACT Square)
  hm  = max(P1' + c13', eps)          (= -p/3 of the depressed cubic)
  q   = (P1' + gh')*(1 - 2 s)         (factorized resolvent q)
  Dm  = hm^3 - q^2/4  (> 0: three real roots always)
  u   = q * 0.5/sqrt(Dm)              (rsqrt via exp(-0.5 ln Dm))
  root= s + 2 sqrt(hm) * sin(arctan(u)/3)
The middle-root identity  cos((phi-2pi)/3) = -sin(arcsin(w)/3)  lets the whole
trig solve use only Arctan and Sin (both in ACT's trig_and_small table set).
Softplus is computed as Ln(1 + Exp(z)) (natural_log_exp set).

Sharding: pure data parallel over batch, 8 cores x 4096 rows.
Host passes x and (x_b + m/gamma) pre-transposed, so the PE contraction dim
(signal index) lands on SBUF partitions with plain contiguous DMA loads.
"""

import numpy as np

B_TOTAL = 32768
N = 256
N_CORES = 8
BC = B_TOTAL // N_CORES      # rows per core
TILES = BC // 128            # 32 batch tiles of 128
CT = 4                       # tiles per elementwise chunk
CHUNKS = TILES // CT         # 8
CF = CT * N                  # chunk free size (1024)
EPS_H = np.float32(1e-12 / 3.0)
USE_F32R = True

_PROG = {}


def _np_f32(a):
    return np.ascontiguousarray(np.asarray(a, dtype=np.float32))


def _conv_pool_mat(w, L):
    """(L/4, L) matrix implementing conv1d(k=5,pad=2) then avgpool4."""
    taps = np.asarray(w, np.float32).reshape(5)
    C = np.zeros((L, L), np.float32)
    for n in range(L):
        for k in range(5):
            m = n + k - 2
            if 0 <= m < L:
                C[n, m] = taps[k]
    P = np.zeros((L // 4, L), np.float32)
    for i in range(L // 4):
        P[i, 4 * i:4 * i + 4] = 0.25
    return (P @ C).astype(np.float32)


_CUSTOM_OPS = {}


def _get_custom_ops():
    """Register this kernel's fused custom-DVE ops (idempotent).

    HM3: h3 = clamp((s-1/2)^2 + c13', eps)^3      (one op: hm and its cube)
    QQ:  q  = ((s-1/2)^2 + gh')*(1 - 2s)          (factorized resolvent)
    DM4: dm4 = 4*h3 - q^2                          (= 4*discriminant-term)
    """
    if _CUSTOM_OPS:
        return _CUSTOM_OPS
    import concourse.dve_ops as dops
    from concourse.dve_spec import (Spec, Src0, Src1, C0, C1, C2, sq, maxx,
                                    lower, _has_src1)
    from concourse.dve_uop import DveOpSpec

    def reg(name, spec):
        if name in dops._SUB_OPCODE_FOR_NAME:
            return next(o for o in dops.OPS if o.name == name)
        row = dops._CUSTOM_DVE_ROW_BASE + len(dops.OPS)
        assert row < 0x20
        dops._SUB_OPCODE_FOR_NAME[name] = row
        shas = {}
        for ver in ("v3", "v4"):
            u = lower(spec, ver=ver)
            shas[ver] = DveOpSpec(name=name, opcode=row, uops=u,
                                  rd1_en=_has_src1(spec)).sha(ver)
        op = dops.DveOp(name, spec, subdim=False, uops_sha=shas)
        dops.OPS.append(op)
        dops.CUSTOM_DVE_SPECS[name] = spec
        return op

    import numpy as np_

    _hm = maxx(sq(Src0 - C2) + C0, C1)
    _CUSTOM_OPS['HM3'] = reg('ANT_K_HM3', Spec(
        body=sq(_hm) * _hm,
        reference=lambda in0, in1, s0, s1, imm2:
            (lambda h: (h * h * h).astype(np_.float32))(
                np_.maximum((in0 - imm2) ** 2 + s0, s1).astype(np_.float32)),
    ))
    _CUSTOM_OPS['QQ'] = reg('ANT_K_QQ', Spec(
        body=(sq(Src0 - C2) + C0) * (C1 - (Src0 + Src0)),
        reference=lambda in0, in1, s0, s1, imm2:
            (((in0 - imm2) ** 2 + s0) * (s1 - (in0 + in0))).astype(np_.float32),
    ))
    _CUSTOM_OPS['DM4'] = reg('ANT_K_DM4', Spec(
        body=Src1 * C0 - sq(Src0),
        reference=lambda in0, in1, s0, s1, imm2:
            (in1 * s0 - in0 ** 2).astype(np_.float32),
    ))
    return _CUSTOM_OPS


_TABLES_PATCHED = False


def _patch_act_tables():
    """Restrict ACT table-set choice to the two sets this kernel uses.

    bacc's set chooser otherwise binds Exp to `exp_and_others` and Ln to
    `natural_log` (first set containing each function), which forces a
    ~1.3us ACT_TABLE_LOAD on every Exp<->Ln transition.  Emptying all other
    sets makes both resolve to `natural_log_exp_and_others` (set ids keep
    their act_info.json indices, so walrus agrees)."""
    global _TABLES_PATCHED
    if _TABLES_PATCHED:
        return
    import concourse.bacc as bacc
    keep = {'natural_log_exp_and_others', 'trig_and_small'}
    orig = bacc.get_activation_tables

    def patched(arch):
        t = orig(arch)
        return {k: (v if k in keep else set()) for k, v in t.items()}

    bacc.get_activation_tables = patched
    _TABLES_PATCHED = True


def _build_program():
    import concourse.bacc as bacc
    import concourse.tile as tile
    import concourse.mybir as mybir
    from concourse.tile import add_dep_helper
    _patch_act_tables()

    dt = mybir.dt
    f32 = dt.float32
    Alu = mybir.AluOpType
    AF = mybir.ActivationFunctionType

    mmdt = dt.float32r if USE_F32R else dt.float32

    def mmcast(ap):
        return ap

    COPS = _get_custom_ops()
    nc = bacc.Bacc("TRN2", target_bir_lowering=False, debug=False,
                   num_devices=N_CORES)

    XT = nc.dram_tensor("xt", (256, BC), mmdt, kind="ExternalInput")
    XB3 = nc.dram_tensor("xb3", (BC, 256), f32, kind="ExternalInput")
    WM = nc.dram_tensor("wm", (256, 512), mmdt, kind="ExternalInput")
    M1T = nc.dram_tensor("m1t", (256, 128), mmdt, kind="ExternalInput")
    M2BD = nc.dram_tensor("m2bd", (128, 32), mmdt, kind="ExternalInput")
    LWBD = nc.dram_tensor("lwbd", (32, 2), mmdt, kind="ExternalInput")
    B2V = nc.dram_tensor("b2v", (128, 1), f32, kind="ExternalInput")
    B3V = nc.dram_tensor("b3v", (32, 1), f32, kind="ExternalInput")
    LBM = nc.dram_tensor("lbm", (128, 1), f32, kind="ExternalInput")
    LBR = nc.dram_tensor("lbr", (128, 1), f32, kind="ExternalInput")
    GSC = nc.dram_tensor("gsc", (128, 1), f32, kind="ExternalInput")
    OUT = nc.dram_tensor("out", (BC, 256), f32, kind="ExternalOutput")

    with tile.TileContext(nc) as tc:
        with (
            tc.tile_pool(name="const", bufs=1) as cpool,
            tc.tile_pool(name="sa", bufs=6) as sapool,
            tc.tile_pool(name="so", bufs=CHUNKS + 2) as sopool,
            tc.tile_pool(name="scr", bufs=10) as scrpool,
            tc.tile_pool(name="pm", bufs=2, space="PSUM") as pmpool,
            tc.tile_pool(name="pc1", bufs=2, space="PSUM") as pc1pool,
            tc.tile_pool(name="pc2", bufs=1, space="PSUM") as pc2pool,
            tc.tile_pool(name="pc3", bufs=1, space="PSUM") as pc3pool,
        ):
            # ---- constants into SBUF ----
            wm = cpool.tile([128, 2, 512], mmdt)
            m1t = cpool.tile([128, 2, 128], mmdt)
            m2bd = cpool.tile([128, 32], mmdt)
            lwbd = cpool.tile([32, 2], mmdt)
            b2v = cpool.tile([128, 1], f32)
            b3v = cpool.tile([32, 1], f32)
            lbm = cpool.tile([128, 1], f32)
            lbr = cpool.tile([128, 1], f32)
            gsc = cpool.tile([128, 1], f32)
            cln2 = cpool.tile([128, 1], f32)
            nc.vector.memset(cln2[:], float(np.log(2.0)))
            spE = cpool.tile([128, 2 * TILES], f32)
            sp = cpool.tile([128, 2 * TILES], f32)
            gph = cpool.tile([128, TILES], f32)
            c13p = cpool.tile([128, TILES], f32)
            for k in range(2):
                nc.sync.dma_start(wm[:, k, :], WM[128 * k:128 * (k + 1), :])
                nc.sync.dma_start(m1t[:, k, :], M1T[128 * k:128 * (k + 1), :])
            nc.sync.dma_start(m2bd[:], M2BD[:])
            nc.sync.dma_start(lwbd[:], LWBD[:])
            nc.sync.dma_start(b2v[:], B2V[:])
            nc.sync.dma_start(b3v[:], B3V[:])
            nc.sync.dma_start(lbm[:], LBM[:])
            nc.sync.dma_start(lbr[:], LBR[:])
            nc.sync.dma_start(gsc[:], GSC[:])

            s_chunks = []
            for c in range(CHUNKS):
                s_chunks.append(sopool.tile([128, CF], f32, tag="so",
                                            name=f"s{c}"))

            with (
                tc.tile_pool(name="xt", bufs=1) as xtpool,
                tc.tile_pool(name="cnn", bufs=3) as cnnpool,
            ):
                # ---- inputs ----
                xt_sb = xtpool.tile([128, 2, BC], mmdt)
                for k in range(2):
                    nc.sync.dma_start(xt_sb[:, k, :],
                                      XT[128 * k:128 * (k + 1), :])

                # ---- A0: CNN -> per-batch mu/reg scalars ----
                NG = TILES // 4               # groups of 512 batch rows
                h2s_groups = []
                for g in range(NG):
                    sl = slice(512 * g, 512 * (g + 1))
                    p1 = pc1pool.tile([128, 512], f32, tag="p1", name=f"p1g{g}")
                    nc.tensor.matmul(p1[:], mmcast(m1t[:, 0, :]),
                                     mmcast(xt_sb[:, 0, sl]),
                                     start=True, stop=False)
                    nc.tensor.matmul(p1[:], mmcast(m1t[:, 1, :]),
                                     mmcast(xt_sb[:, 1, sl]),
                                     start=False, stop=True)
                    eh1 = cnnpool.tile([128, 512], f32, tag="eh1",
                                       name=f"eh1g{g}")
                    nc.scalar.activation(eh1[:], p1[:], AF.Exp, bias=b2v[:])
                    h1s = cnnpool.tile([128, 512], mmdt, tag="h1s",
                                       name=f"h1sg{g}")
                    nc.scalar.activation(h1s[:], eh1[:], AF.Ln, bias=1.0)
                    p2 = pc2pool.tile([32, 512], f32, tag="p2", name=f"p2g{g}")
                    nc.tensor.matmul(p2[:], mmcast(m2bd[:]), mmcast(h1s[:]),
                                     start=True, stop=True)
                    eh2 = cnnpool.tile([32, 512], f32, tag="eh2",
                                       name=f"eh2g{g}")
                    nc.scalar.activation(eh2[:], p2[:], AF.Exp, bias=b3v[:])
                    h2s = cnnpool.tile([32, 512], mmdt, tag="h2s",
                                       name=f"h2sg{g}")
                    nc.scalar.activation(h2s[:], eh2[:], AF.Ln, bias=1.0)
                    h2s_groups.append(h2s)

                p3 = pc3pool.tile([128, 2 * TILES], f32)
                for t in range(TILES):
                    h2s = h2s_groups[t // 4]
                    nc.tensor.matmul(p3[:, 2 * t:2 * t + 2],
                                     mmcast(h2s[:, 128 * (t % 4):
                                                128 * (t % 4 + 1)]),
                                     mmcast(lwbd[:]), start=True, stop=True)
                p3v = p3[:].rearrange("p (t c) -> p c t", c=2)
                spEv = spE[:].rearrange("p (t c) -> p c t", c=2)
                nc.scalar.activation(spEv[:, 0, :], p3v[:, 0, :], AF.Exp,
                                     bias=lbm[:])
                nc.scalar.activation(spEv[:, 1, :], p3v[:, 1, :], AF.Exp,
                                     bias=lbr[:])
                nc.scalar.activation(sp[:], spE[:], AF.Ln, bias=1.0)
                spv = sp[:].rearrange("p (t c) -> p c t", c=2)
                # gamma_hat' = gamma/m^2 * mu - 1/4 ; c13' = 2/3*gh' + 1/4
                nc.vector.tensor_scalar(gph[:], spv[:, 0, :], gsc[:], -0.25,
                                        Alu.mult, Alu.add)
                nc.vector.tensor_scalar(c13p[:], gph[:], 2.0 / 3.0, 0.25,
                                        Alu.mult, Alu.add)

                # ---- A: main matmuls -> s (two tiles per PSUM pair) ----
                for j in range(TILES // 2):
                    pm = pmpool.tile([128, 2, 512], f32, tag="pm",
                                     name=f"pm{j}")
                    for i in range(2):
                        t = 2 * j + i
                        tsl = slice(128 * t, 128 * (t + 1))
                        nc.tensor.matmul(pm[:, i, :],
                                         mmcast(xt_sb[:, 0, tsl]),
                                         mmcast(wm[:, 0, :]),
                                         start=True, stop=False)
                        nc.tensor.matmul(pm[:, i, :],
                                         mmcast(xt_sb[:, 1, tsl]),
                                         mmcast(wm[:, 1, :]),
                                         start=False, stop=True)
                    sa = sapool.tile([128, 2, 256], f32, tag="sa",
                                     name=f"sa{j}")
                    nc.vector.tensor_copy(sa[:], pm[:, :, 0:256])
                    nc.gpsimd.dma_start(
                        sa[:],
                        XB3[256 * j:256 * (j + 1), :].rearrange(
                            "(i p) n -> p i n", p=128),
                        accum_op=Alu.add)
                    for i in range(2):
                        t = 2 * j + i
                        sc = s_chunks[t // CT]
                        osl = slice(256 * (t % CT), 256 * (t % CT + 1))
                        nc.vector.scalar_tensor_tensor(
                            sc[:, osl], pm[:, i, 256:512],
                            spv[:, 1, t:t + 1], sa[:, i, :],
                            Alu.mult, Alu.add)
                    if t % CT == CT - 1:
                        c = t // CT
                        dview = OUT[512 * c:512 * (c + 1), :].rearrange(
                            "(tt p) n -> p tt n", p=128)
                        nc.sync.dma_start(
                            dview,
                            s_chunks[c][:].rearrange("p (tt n) -> p tt n",
                                                     n=256))

            # ---- B: elementwise cardan chain ----
            with (
                tc.tile_pool(name="ug", bufs=CHUNKS + 1) as ugpool,
                tc.tile_pool(name="hr", bufs=CHUNKS) as hrpool,
            ):
                r_chunks = [None] * CHUNKS
                u_chunks = [None] * CHUNKS
                last_irs = None
                for c in range(CHUNKS):
                    s_c = s_chunks[c]
                    h3 = scrpool.tile([128, CF], f32, tag="scr",
                                      name=f"h3{c}")
                    q = scrpool.tile([128, CF], f32, tag="scr", name=f"q{c}")
                    for i in range(CT):
                        t = CT * c + i
                        osl = slice(256 * i, 256 * (i + 1))
                        nc.vector._custom_dve(
                            COPS['HM3'], out=h3[:, osl], in0=s_c[:, osl],
                            s0=c13p[:, t:t + 1], s1=float(EPS_H), imm2=0.5)
                        nc.vector._custom_dve(
                            COPS['QQ'], out=q[:, osl], in0=s_c[:, osl],
                            s0=gph[:, t:t + 1], s1=1.0, imm2=0.5)
                    dm4 = scrpool.tile([128, CF], f32, tag="scr",
                                       name=f"dm4{c}")
                    nc.vector._custom_dve(COPS['DM4'], out=dm4[:], in0=q[:],
                                          in1=h3[:], s0=4.0)
                    lh = scrpool.tile([128, CF], f32, tag="scr",
                                      name=f"lh{c}")
                    nc.scalar.activation(lh[:], h3[:], AF.Ln)
                    r = hrpool.tile([128, CF], f32, tag="hr", name=f"r{c}")
                    r_chunks[c] = r
                    nc.scalar.activation(r[:], lh[:], AF.Exp,
                                         scale=1.0 / 6.0, bias=cln2[:])
                    ll = scrpool.tile([128, CF], f32, tag="scr",
                                      name=f"ll{c}")
                    nc.scalar.activation(ll[:], dm4[:], AF.Ln)
                    irs = scrpool.tile([128, CF], f32, tag="scr",
                                       name=f"irs{c}")
                    last_irs = nc.scalar.activation(irs[:], ll[:], AF.Exp,
                                                    scale=-0.5)
                    u = ugpool.tile([128, CF], f32, tag="ug", name=f"u{c}")
                    u_chunks[c] = u
                    nc.vector.tensor_tensor(u[:], q[:], irs[:], Alu.mult)

                # trig phase (one table load; force ACT order after NLE)
                for c in range(CHUNKS):
                    at = scrpool.tile([128, CF], f32, tag="scr",
                                      name=f"at{c}")
                    at_i = nc.scalar.activation(at[:], u_chunks[c][:],
                                                AF.Arctan)
                    if c == 0:
                        add_dep_helper(at_i.ins, last_irs.ins, sync=False,
                                       reason="keep trig table load last")
                    gg = ugpool.tile([128, CF], f32, tag="ug", name=f"g{c}")
                    nc.scalar.activation(gg[:], at[:], AF.Sin,
                                         scale=1.0 / 3.0)
                    rg = scrpool.tile([128, CF], f32, tag="scr",
                                      name=f"rg{c}")
                    nc.gpsimd.tensor_tensor(rg[:], r_chunks[c][:], gg[:],
                                            Alu.mult)
                    dview = OUT[512 * c:512 * (c + 1), :].rearrange(
                        "(tt p) n -> p tt n", p=128)
                    nc.gpsimd.dma_start(
                        dview,
                        rg[:].rearrange("p (tt n) -> p tt n", n=256),
                        accum_op=Alu.add)

    nc.compile()
    return nc


def _get_program():
    key = (B_TOTAL, N, N_CORES, USE_F32R)
    if key not in _PROG:
        _PROG[key] = _build_program()
    return _PROG[key]


def _host_prep(inputs):
    x = _np_f32(inputs['x']).reshape(B_TOTAL, N)
    x_b = _np_f32(inputs['x_b']).reshape(B_TOTAL, N)
    m = float(np.asarray(inputs['mass']).reshape(-1)[0])
    gp = float(np.asarray(inputs['gamma_p']).reshape(-1)[0])
    # softplus in float64 for the scalar
    gamma = float(np.log1p(np.exp(gp))) if gp < 30 else gp
    TtT = _np_f32(inputs['TtT'])
    DtD = _np_f32(inputs['DtD'])

    W_A = ((np.eye(N, dtype=np.float32) - np.float32(gamma) * TtT.T)
           / np.float32(3.0 * m)).astype(np.float32)
    W_B = (-np.float32(gamma) * DtD.T / np.float32(3.0 * m)).astype(np.float32)
    WM = np.ascontiguousarray(np.concatenate([W_A, W_B], axis=1))     # (256,512)

    M1s, M2s, lws = {}, {}, {}
    for tag in ('mu', 'reg'):
        M1s[tag] = _conv_pool_mat(inputs['w2_' + tag], 256)            # (64,256)
        M2s[tag] = _conv_pool_mat(inputs['w3_' + tag], 64)             # (16,64)
        lws[tag] = _np_f32(inputs['lw_' + tag]).reshape(16)
    M1cat = np.concatenate([M1s['mu'], M1s['reg']], axis=0)            # (128,256)
    M1T = np.ascontiguousarray(M1cat.T)                                # (256,128)
    M2BD = np.zeros((128, 32), np.float32)
    M2BD[0:64, 0:16] = M2s['mu'].T
    M2BD[64:128, 16:32] = M2s['reg'].T
    LWBD = np.zeros((32, 2), np.float32)
    LWBD[0:16, 0] = lws['mu']
    LWBD[16:32, 1] = lws['reg']

    def sc(name):
        return float(np.asarray(inputs[name]).reshape(-1)[0])

    B2V = np.full((128, 1), sc('b2_mu'), np.float32)
    B2V[64:] = sc('b2_reg')
    B3V = np.full((32, 1), sc('b3_mu'), np.float32)
    B3V[16:] = sc('b3_reg')
    LBM = np.full((128, 1), sc('lb_mu'), np.float32)
    LBR = np.full((128, 1), sc('lb_reg'), np.float32)
    GSC = np.full((128, 1), gamma / (m * m), np.float32)

    consts = dict(wm=WM, m1t=M1T, m2bd=M2BD, lwbd=LWBD,
                  b2v=B2V, b3v=B3V, lbm=LBM, lbr=LBR, gsc=GSC)

    xb3 = (np.float32(gamma / (3.0 * m)) * x_b
           + np.float32(1.0 / 3.0)).astype(np.float32)
    in_maps = []
    for c in range(N_CORES):
        rows = slice(BC * c, BC * (c + 1))
        im = dict(consts)
        im['xt'] = np.ascontiguousarray(x[rows].T)
        im['xb3'] = np.ascontiguousarray(xb3[rows])
        in_maps.append(im)
    return in_maps, m


def kernel(**inputs) -> np.ndarray:
    from concourse import bass_utils
    nc = _get_program()
    in_maps, m = _host_prep(inputs)
    res = bass_utils.run_bass_kernel_spmd(nc, in_maps,
                                          core_ids=list(range(N_CORES)))
    out = np.concatenate([res.results[c]['out'] for c in range(N_CORES)],
                         axis=0)
    if m != 1.0:
        out = (np.float32(m) * out).astype(np.float32)
    return np.ascontiguousarray(out.reshape(B_TOTAL, 1, N))


# revision 14
# speedup vs baseline: 1.2204x; 1.2204x over previous
"""Trainium2 Bass kernel for nn_Block_34067680592489.

Computes, for B=32768 independent signals x[b] (length 256):
  mu,reg = small-CNN(x[b])      (conv5+avgpool4+softplus twice, linear, softplus)
  grad   = TtT x - x_b + reg * DtD x
  x_t    = x - gamma * grad,  gamma = softplus(gamma_p)
  out    = middle root of z^3 -(m+x_t) z^2 + (m x_t - 2 gm) z + gm m,  gm = gamma*mu

Device algorithm (per element, after normalizing the cubic to mass=1):
  s   = (1 + xt)/3                    (from PE matmuls, accumulated in PSUM)
  P1' = (s - 1/2)^2                   (ACT Square)
  hm  = max(P1' + c13', eps)          (= -p/3 of the depressed cubic)
  q   = (P1' + gh')*(1 - 2 s)         (factorized resolvent q)
  Dm  = hm^3 - q^2/4  (> 0: three real roots always)
  u   = q * 0.5/sqrt(Dm)              (rsqrt via exp(-0.5 ln Dm))
  root= s + 2 sqrt(hm) * sin(arctan(u)/3)
The middle-root identity  cos((phi-2pi)/3) = -sin(arcsin(w)/3)  lets the whole
trig solve use only Arctan and Sin (both in ACT's trig_and_small table set).
Softplus is computed as Ln(1 + Exp(z)) (natural_log_exp set).

Sharding: pure data parallel over batch, 8 cores x 4096 rows.
Host passes x and (x_b + m/gamma) pre-transposed, so the PE contraction dim
(signal index) lands on SBUF partitions with plain contiguous DMA loads.
"""

import numpy as np

B_TOTAL = 32768
N = 256
N_CORES = 8
BC = B_TOTAL // N_CORES      # rows per core
TILES = BC // 128            # 32 batch tiles of 128
CT = 4                       # tiles per elementwise chunk
CHUNKS = TILES // CT         # 8
CF = CT * N                  # chunk free size (1024)
EPS_H = np.float32(1e-12 / 3.0)
USE_F32R = True

_PROG = {}


def _np_f32(a):
    return np.ascontiguousarray(np.asarray(a, dtype=np.float32))


def _conv_pool_mat(w, L):
    """(L/4, L) matrix implementing conv1d(k=5,pad=2) then avgpool4."""
    taps = np.asarray(w, np.float32).reshape(5)
    C = np.zeros((L, L), np.float32)
    for n in range(L):
        for k in range(5):
            m = n + k - 2
            if 0 <= m < L:
                C[n, m] = taps[k]
    P = np.zeros((L // 4, L), np.float32)
    for i in range(L // 4):
        P[i, 4 * i:4 * i + 4] = 0.25
    return (P @ C).astype(np.float32)


_CUSTOM_OPS = {}


def _get_custom_ops():
    """Register this kernel's fused custom-DVE ops (idempotent).

    HM3: h3 = clamp((s-1/2)^2 + c13', eps)^3      (one op: hm and its cube)
    QQ:  q  = ((s-1/2)^2 + gh')*(1 - 2s)          (factorized resolvent)
    DM4: dm4 = 4*h3 - q^2                          (= 4*discriminant-term)
    """
    if _CUSTOM_OPS:
        return _CUSTOM_OPS
    import concourse.dve_ops as dops
    from concourse.dve_spec import (Spec, Src0, Src1, C0, C1, C2, sq, maxx,
                                    lower, _has_src1)
    from concourse.dve_uop import DveOpSpec

    def reg(name, spec):
        if name in dops._SUB_OPCODE_FOR_NAME:
            return next(o for o in dops.OPS if o.name == name)
        row = dops._CUSTOM_DVE_ROW_BASE + len(dops.OPS)
        assert row < 0x20
        dops._SUB_OPCODE_FOR_NAME[name] = row
        shas = {}
        for ver in ("v3", "v4"):
            u = lower(spec, ver=ver)
            shas[ver] = DveOpSpec(name=name, opcode=row, uops=u,
                                  rd1_en=_has_src1(spec)).sha(ver)
        op = dops.DveOp(name, spec, subdim=False, uops_sha=shas)
        dops.OPS.append(op)
        dops.CUSTOM_DVE_SPECS[name] = spec
        return op

    import numpy as np_

    _hm = maxx(sq(Src0 - C2) + C0, C1)
    _CUSTOM_OPS['HM3'] = reg('ANT_K_HM3', Spec(
        body=sq(_hm) * _hm,
        reference=lambda in0, in1, s0, s1, imm2:
            (lambda h: (h * h * h).astype(np_.float32))(
                np_.maximum((in0 - imm2) ** 2 + s0, s1).astype(np_.float32)),
    ))
    _CUSTOM_OPS['QQ'] = reg('ANT_K_QQ', Spec(
        body=(sq(Src0 - C2) + C0) * (C1 - (Src0 + Src0)),
        reference=lambda in0, in1, s0, s1, imm2:
            (((in0 - imm2) ** 2 + s0) * (s1 - (in0 + in0))).astype(np_.float32),
    ))
    _CUSTOM_OPS['DM4'] = reg('ANT_K_DM4', Spec(
        body=Src1 * C0 - sq(Src0),
        reference=lambda in0, in1, s0, s1, imm2:
            (in1 * s0 - in0 ** 2).astype(np_.float32),
    ))
    return _CUSTOM_OPS


_TABLES_PATCHED = False


def _patch_act_tables():
    """Restrict ACT table-set choice to the two sets this kernel uses.

    bacc's set chooser otherwise binds Exp to `exp_and_others` and Ln to
    `natural_log` (first set containing each function), which forces a
    ~1.3us ACT_TABLE_LOAD on every Exp<->Ln transition.  Emptying all other
    sets makes both resolve to `natural_log_exp_and_others` (set ids keep
    their act_info.json indices, so walrus agrees)."""
    global _TABLES_PATCHED
    if _TABLES_PATCHED:
        return
    import concourse.bacc as bacc
    keep = {'natural_log_exp_and_others', 'trig_and_small'}
    orig = bacc.get_activation_tables

    def patched(arch):
        t = orig(arch)
        return {k: (v if k in keep else set()) for k, v in t.items()}

    bacc.get_activation_tables = patched
    _TABLES_PATCHED = True


def _build_program():
    import concourse.bacc as bacc
    import concourse.tile as tile
    import concourse.mybir as mybir
    from concourse.tile import add_dep_helper
    _patch_act_tables()

    dt = mybir.dt
    f32 = dt.float32
    Alu = mybir.AluOpType
    AF = mybir.ActivationFunctionType

    mmdt = dt.float32r if USE_F32R else dt.float32

    def mmcast(ap):
        return ap

    COPS = _get_custom_ops()
    nc = bacc.Bacc("TRN2", target_bir_lowering=False, debug=False,
                   num_devices=N_CORES)

    XT = nc.dram_tensor("xt", (256, BC), mmdt, kind="ExternalInput")
    XB3 = nc.dram_tensor("xb3", (BC, 256), f32, kind="ExternalInput")
    WM = nc.dram_tensor("wm", (256, 512), mmdt, kind="ExternalInput")
    M1T = nc.dram_tensor("m1t", (256, 128), mmdt, kind="ExternalInput")
    M2BD = nc.dram_tensor("m2bd", (128, 32), mmdt, kind="ExternalInput")
    LWBD = nc.dram_tensor("lwbd", (32, 2), mmdt, kind="ExternalInput")
    B2V = nc.dram_tensor("b2v", (128, 1), f32, kind="ExternalInput")
    B3V = nc.dram_tensor("b3v", (32, 1), f32, kind="ExternalInput")
    LBM = nc.dram_tensor("lbm", (128, 1), f32, kind="ExternalInput")
    LBR = nc.dram_tensor("lbr", (128, 1), f32, kind="ExternalInput")
    GSC = nc.dram_tensor("gsc", (128, 1), f32, kind="ExternalInput")
    OUT = nc.dram_tensor("out", (BC, 256), f32, kind="ExternalOutput")

    with tile.TileContext(nc) as tc:
        with (
            tc.tile_pool(name="const", bufs=1) as cpool,
            tc.tile_pool(name="sa", bufs=6) as sapool,
            tc.tile_pool(name="so", bufs=CHUNKS + 2) as sopool,
            tc.tile_pool(name="scr", bufs=10) as scrpool,
            tc.tile_pool(name="pm", bufs=2, space="PSUM") as pmpool,
            tc.tile_pool(name="pc1", bufs=2, space="PSUM") as pc1pool,
            tc.tile_pool(name="pc2", bufs=1, space="PSUM") as pc2pool,
            tc.tile_pool(name="pc3", bufs=1, space="PSUM") as pc3pool,
        ):
            # ---- constants into SBUF ----
            wm = cpool.tile([128, 2, 512], mmdt)
            m1t = cpool.tile([128, 2, 128], mmdt)
            m2bd = cpool.tile([128, 32], mmdt)
            lwbd = cpool.tile([32, 2], mmdt)
            b2v = cpool.tile([128, 1], f32)
            b3v = cpool.tile([32, 1], f32)
            lbm = cpool.tile([128, 1], f32)
            lbr = cpool.tile([128, 1], f32)
            gsc = cpool.tile([128, 1], f32)
            cln2 = cpool.tile([128, 1], f32)
            nc.vector.memset(cln2[:], float(np.log(2.0)))
            spE = cpool.tile([128, 2 * TILES], f32)
            sp = cpool.tile([128, 2 * TILES], f32)
            gph = cpool.tile([128, TILES], f32)
            c13p = cpool.tile([128, TILES], f32)
            for k in range(2):
                nc.sync.dma_start(wm[:, k, :], WM[128 * k:128 * (k + 1), :])
                nc.sync.dma_start(m1t[:, k, :], M1T[128 * k:128 * (k + 1), :])
            nc.sync.dma_start(m2bd[:], M2BD[:])
            nc.sync.dma_start(lwbd[:], LWBD[:])
            nc.sync.dma_start(b2v[:], B2V[:])
            nc.sync.dma_start(b3v[:], B3V[:])
            nc.sync.dma_start(lbm[:], LBM[:])
            nc.sync.dma_start(lbr[:], LBR[:])
            nc.sync.dma_start(gsc[:], GSC[:])

            s_chunks = []
            for c in range(CHUNKS):
                s_chunks.append(sopool.tile([128, CF], f32, tag="so",
                                            name=f"s{c}"))

            with (
                tc.tile_pool(name="xt", bufs=1) as xtpool,
                tc.tile_pool(name="cnn", bufs=3) as cnnpool,
            ):
                # ---- inputs ----
                xt_sb = xtpool.tile([128, 2, BC], mmdt)
                for k in range(2):
                    nc.sync.dma_start(xt_sb[:, k, :],
                                      XT[128 * k:128 * (k + 1), :])

                # ---- A0: CNN -> per-batch mu/reg scalars ----
                NG = TILES // 4               # groups of 512 batch rows
                h2s_groups = []
                for g in range(NG):
                    sl = slice(512 * g, 512 * (g + 1))
                    p1 = pc1pool.tile([128, 512], f32, tag="p1", name=f"p1g{g}")
                    nc.tensor.matmul(p1[:], mmcast(m1t[:, 0, :]),
                                     mmcast(xt_sb[:, 0, sl]),
                                     start=True, stop=False)
                    nc.tensor.matmul(p1[:], mmcast(m1t[:, 1, :]),
                                     mmcast(xt_sb[:, 1, sl]),
                                     start=False, stop=True)
                    eh1 = cnnpool.tile([128, 512], f32, tag="eh1",
                                       name=f"eh1g{g}")
                    nc.scalar.activation(eh1[:], p1[:], AF.Exp, bias=b2v[:])
                    h1s = cnnpool.tile([128, 512], mmdt, tag="h1s",
                                       name=f"h1sg{g}")
                    nc.scalar.activation(h1s[:], eh1[:], AF.Ln, bias=1.0)
                    p2 = pc2pool.tile([32, 512], f32, tag="p2", name=f"p2g{g}")
                    nc.tensor.matmul(p2[:], mmcast(m2bd[:]), mmcast(h1s[:]),
                                     start=True, stop=True)
                    eh2 = cnnpool.tile([32, 512], f32, tag="eh2",
                                       name=f"eh2g{g}")
                    nc.scalar.activation(eh2[:], p2[:], AF.Exp, bias=b3v[:])
                    h2s = cnnpool.tile([32, 512], mmdt, tag="h2s",
                                       name=f"h2sg{g}")
                    nc.scalar.activation(h2s[:], eh2[:], AF.Ln, bias=1.0)
                    h2s_groups.append(h2s)

                p3 = pc3pool.tile([128, 2 * TILES], f32)
                for t in range(TILES):
                    h2s = h2s_groups[t // 4]
                    nc.tensor.matmul(p3[:, 2 * t:2 * t + 2],
                                     mmcast(h2s[:, 128 * (t % 4):
                                                128 * (t % 4 + 1)]),
                                     mmcast(lwbd[:]), start=True, stop=True)
                p3v = p3[:].rearrange("p (t c) -> p c t", c=2)
                spEv = spE[:].rearrange("p (t c) -> p c t", c=2)
                nc.scalar.activation(spEv[:, 0, :], p3v[:, 0, :], AF.Exp,
                                     bias=lbm[:])
                nc.scalar.activation(spEv[:, 1, :], p3v[:, 1, :], AF.Exp,
                                     bias=lbr[:])
                nc.scalar.activation(sp[:], spE[:], AF.Ln, bias=1.0)
                spv = sp[:].rearrange("p (t c) -> p c t", c=2)
                # gamma_hat' = gamma/m^2 * mu - 1/4 ; c13' = 2/3*gh' + 1/4
                nc.vector.tensor_scalar(gph[:], spv[:, 0, :], gsc[:], -0.25,
                                        Alu.mult, Alu.add)
                nc.vector.tensor_scalar(c13p[:], gph[:], 2.0 / 3.0, 0.25,
                                        Alu.mult, Alu.add)

                # ---- A: main matmuls -> s (two tiles per PSUM pair) ----
                for j in range(TILES // 2):
                    pm = pmpool.tile([128, 2, 512], f32, tag="pm",
                                     name=f"pm{j}")
                    for i in range(2):
                        t = 2 * j + i
                        tsl = slice(128 * t, 128 * (t + 1))
                        nc.tensor.matmul(pm[:, i, :],
                                         mmcast(xt_sb[:, 0, tsl]),
                                         mmcast(wm[:, 0, :]),
                                         start=True, stop=False)
                        nc.tensor.matmul(pm[:, i, :],
                                         mmcast(xt_sb[:, 1, tsl]),
                                         mmcast(wm[:, 1, :]),
                                         start=False, stop=True)
                    sa = sapool.tile([128, 2, 256], f32, tag="sa",
                                     name=f"sa{j}")
                    nc.vector.tensor_copy(sa[:], pm[:, :, 0:256])
                    for i in range(2):
                        t = 2 * j + i
                        sc = s_chunks[t // CT]
                        osl = slice(256 * (t % CT), 256 * (t % CT + 1))
                        nc.vector.scalar_tensor_tensor(
                            sc[:, osl], pm[:, i, 256:512],
                            spv[:, 1, t:t + 1], sa[:, i, :],
                            Alu.mult, Alu.add)
                    if t % CT == CT - 1:
                        c = t // CT
                        nc.gpsimd.dma_start(
                            s_chunks[c][:].rearrange("p (tt n) -> p tt n",
                                                     n=256),
                            XB3[512 * c:512 * (c + 1), :].rearrange(
                                "(tt p) n -> p tt n", p=128),
                            accum_op=Alu.add)
                        dview = OUT[512 * c:512 * (c + 1), :].rearrange(
                            "(tt p) n -> p tt n", p=128)
                        nc.sync.dma_start(
                            dview,
                            s_chunks[c][:].rearrange("p (tt n) -> p tt n",
                                                     n=256))

            # ---- B: elementwise cardan chain ----
            with (
                tc.tile_pool(name="ug", bufs=CHUNKS + 1) as ugpool,
                tc.tile_pool(name="hr", bufs=CHUNKS) as hrpool,
            ):
                r_chunks = [None] * CHUNKS
                u_chunks = [None] * CHUNKS
                prev_tail = None
                for half in range(2):
                    cs = range(half * CHUNKS // 2, (half + 1) * CHUNKS // 2)
                    last_irs = None
                    first_lh = None
                    for c in cs:
                        s_c = s_chunks[c]
                        h3 = scrpool.tile([128, CF], f32, tag="scr",
                                          name=f"h3{c}")
                        q = scrpool.tile([128, CF], f32, tag="scr",
                                         name=f"q{c}")
                        for i in range(CT):
                            t = CT * c + i
                            osl = slice(256 * i, 256 * (i + 1))
                            nc.vector._custom_dve(
                                COPS['HM3'], out=h3[:, osl],
                                in0=s_c[:, osl],
                                s0=c13p[:, t:t + 1], s1=float(EPS_H),
                                imm2=0.5)
                            nc.vector._custom_dve(
                                COPS['QQ'], out=q[:, osl], in0=s_c[:, osl],
                                s0=gph[:, t:t + 1], s1=1.0, imm2=0.5)
                        dm4 = scrpool.tile([128, CF], f32, tag="scr",
                                           name=f"dm4{c}")
                        nc.vector._custom_dve(COPS['DM4'], out=dm4[:],
                                              in0=q[:], in1=h3[:], s0=4.0)
                        lh = scrpool.tile([128, CF], f32, tag="scr",
                                          name=f"lh{c}")
                        lh_i = nc.scalar.activation(lh[:], h3[:], AF.Ln)
                        if first_lh is None:
                            first_lh = lh_i
                        r = hrpool.tile([128, CF], f32, tag="hr",
                                        name=f"r{c}")
                        r_chunks[c] = r
                        nc.scalar.activation(r[:], lh[:], AF.Exp,
                                             scale=1.0 / 6.0, bias=cln2[:])
                        ll = scrpool.tile([128, CF], f32, tag="scr",
                                          name=f"ll{c}")
                        nc.scalar.activation(ll[:], dm4[:], AF.Ln)
                        irs = scrpool.tile([128, CF], f32, tag="scr",
                                           name=f"irs{c}")
                        last_irs = nc.scalar.activation(irs[:], ll[:],
                                                        AF.Exp, scale=-0.5)
                        u = ugpool.tile([128, CF], f32, tag="ug",
                                        name=f"u{c}")
                        u_chunks[c] = u
                        nc.gpsimd.tensor_tensor(u[:], q[:], irs[:],
                                                Alu.mult)
                    if prev_tail is not None:
                        add_dep_helper(first_lh.ins, prev_tail.ins,
                                       sync=False,
                                       reason="NLE after prior trig half")
                    # trig + tail for this half
                    for c in cs:
                        at = scrpool.tile([128, CF], f32, tag="scr",
                                          name=f"at{c}")
                        at_i = nc.scalar.activation(at[:], u_chunks[c][:],
                                                    AF.Arctan)
                        if c == cs[0]:
                            add_dep_helper(at_i.ins, last_irs.ins,
                                           sync=False,
                                           reason="trig after NLE half")
                        gg = ugpool.tile([128, CF], f32, tag="ug",
                                         name=f"g{c}")
                        prev_tail = nc.scalar.activation(gg[:], at[:],
                                                         AF.Sin,
                                                         scale=1.0 / 3.0)
                        rg = scrpool.tile([128, CF], f32, tag="scr",
                                          name=f"rg{c}")
                        nc.vector.tensor_tensor(rg[:], r_chunks[c][:],
                                                gg[:], Alu.mult)
                        dview = OUT[512 * c:512 * (c + 1), :].rearrange(
                            "(tt p) n -> p tt n", p=128)
                        nc.gpsimd.dma_start(
                            dview,
                            rg[:].rearrange("p (tt n) -> p tt n", n=256),
                            accum_op=Alu.add)

    nc.compile()
    return nc


def _get_program():
    key = (B_TOTAL, N, N_CORES, USE_F32R)
    if key not in _PROG:
        _PROG[key] = _build_program()
    return _PROG[key]


def _host_prep(inputs):
    x = _np_f32(inputs['x']).reshape(B_TOTAL, N)
    x_b = _np_f32(inputs['x_b']).reshape(B_TOTAL, N)
    m = float(np.asarray(inputs['mass']).reshape(-1)[0])
    gp = float(np.asarray(inputs['gamma_p']).reshape(-1)[0])
    # softplus in float64 for the scalar
    gamma = float(np.log1p(np.exp(gp))) if gp < 30 else gp
    TtT = _np_f32(inputs['TtT'])
    DtD = _np_f32(inputs['DtD'])

    W_A = ((np.eye(N, dtype=np.float32) - np.float32(gamma) * TtT.T)
           / np.float32(3.0 * m)).astype(np.float32)
    W_B = (-np.float32(gamma) * DtD.T / np.float32(3.0 * m)).astype(np.float32)
    WM = np.ascontiguousarray(np.concatenate([W_A, W_B], axis=1))     # (256,512)

    M1s, M2s, lws = {}, {}, {}
    for tag in ('mu', 'reg'):
        M1s[tag] = _conv_pool_mat(inputs['w2_' + tag], 256)            # (64,256)
        M2s[tag] = _conv_pool_mat(inputs['w3_' + tag], 64)             # (16,64)
        lws[tag] = _np_f32(inputs['lw_' + tag]).reshape(16)
    M1cat = np.concatenate([M1s['mu'], M1s['reg']], axis=0)            # (128,256)
    M1T = np.ascontiguousarray(M1cat.T)                                # (256,128)
    M2BD = np.zeros((128, 32), np.float32)
    M2BD[0:64, 0:16] = M2s['mu'].T
    M2BD[64:128, 16:32] = M2s['reg'].T
    LWBD = np.zeros((32, 2), np.float32)
    LWBD[0:16, 0] = lws['mu']
    LWBD[16:32, 1] = lws['reg']

    def sc(name):
        return float(np.asarray(inputs[name]).reshape(-1)[0])

    B2V = np.full((128, 1), sc('b2_mu'), np.float32)
    B2V[64:] = sc('b2_reg')
    B3V = np.full((32, 1), sc('b3_mu'), np.float32)
    B3V[16:] = sc('b3_reg')
    LBM = np.full((128, 1), sc('lb_mu'), np.float32)
    LBR = np.full((128, 1), sc('lb_reg'), np.float32)
    GSC = np.full((128, 1), gamma / (m * m), np.float32)

    consts = dict(wm=WM, m1t=M1T, m2bd=M2BD, lwbd=LWBD,
                  b2v=B2V, b3v=B3V, lbm=LBM, lbr=LBR, gsc=GSC)

    xb3 = (np.float32(gamma / (3.0 * m)) * x_b
           + np.float32(1.0 / 3.0)).astype(np.float32)
    in_maps = []
    for c in range(N_CORES):
        rows = slice(BC * c, BC * (c + 1))
        im = dict(consts)
        im['xt'] = np.ascontiguousarray(x[rows].T)
        im['xb3'] = np.ascontiguousarray(xb3[rows])
        in_maps.append(im)
    return in_maps, m


def kernel(**inputs) -> np.ndarray:
    from concourse import bass_utils
    nc = _get_program()
    in_maps, m = _host_prep(inputs)
    res = bass_utils.run_bass_kernel_spmd(nc, in_maps,
                                          core_ids=list(range(N_CORES)))
    out = np.concatenate([res.results[c]['out'] for c in range(N_CORES)],
                         axis=0)
    if m != 1.0:
        out = (np.float32(m) * out).astype(np.float32)
    return np.ascontiguousarray(out.reshape(B_TOTAL, 1, N))
